# revision 6
# baseline (speedup 1.0000x reference)
"""Trainium2 Bass kernel for the latent-type LSTM LM (nn_LM_latent_type_rep).

Strategy (8 NeuronCores, SPMD):
  - LSTM recurrence is model-parallel over the 4H gate dimension: each core
    owns 128 rows of each of the i,f,g,o gate blocks (512 gate rows) and the
    matching 128-dim slice of h/c. Per step, h-shards are exchanged with an
    AllGather (bf16, [128,16] per core).
  - The tag head is folded into the recurrence weights: tag_log = h @ (W_tag
    @ W_lh).T + (b_lh @ W_tag.T + b_tag); computed replicated on every core
    as a 5th matmul tile over the gathered h.  The tag feedback into the
    gates uses M_tag = W_tag @ W_ih[:, E_TOK:].T (a weight-only fold).
  - The token-embedding contribution X = emb @ W_ih[:, :E_TOK].T is computed
    on device as one large matmul in a preamble.
  - The word projection (the bulk of FLOPs + output bytes) is sharded over
    the NT*TV=32000 projection rows: 4000 rows/core; computed in chunks of
    32 timesteps (512 tokens) interleaved into the recurrence's AllGather
    wait gaps.
"""

import numpy as np
import ml_dtypes

import concourse.bass as bass
import concourse.bacc as bacc
import concourse.tile as tile
import concourse.mybir as mybir
from concourse.bass_utils import run_bass_kernel_spmd

AF = mybir.ActivationFunctionType
ALU = mybir.AluOpType
F32 = mybir.dt.float32
BF16 = mybir.dt.bfloat16

B = 16
S = 256
VOCAB = 50000
H = 1024
E_TOK = 512
E_TAG = 128
NT = 4
TV = 8000
NC = 8           # cores
GR = 512         # gate rows per core (4 gates x 128)
HS = 128         # h-slice per core
PR = NT * TV // NC  # projection rows per core = 4000
CHUNK = 32       # timesteps per projection chunk
NB = 500         # projection N per matmul (4000 = 8*500)

bf16 = ml_dtypes.bfloat16


def build_kernel(s_steps=S, chunk=None):
    """Build the SPMD Bass kernel. Returns (nc, names dict)."""
    SN = s_steps
    NTOK = SN * B
    if chunk is None:
        chunk = min(CHUNK, SN)
    n_chunks = SN // chunk
    assert SN % chunk == 0
    CH16 = chunk * B  # columns per chunk of hT (512)

    nc = bacc.Bacc(None, target_bir_lowering=False, num_devices=NC)

    # ---------------- DRAM I/O ----------------
    embT_d = nc.dram_tensor("embT", [128, 4, NTOK], BF16, kind="ExternalInput")
    wx_d = nc.dram_tensor("wxT", [128, 4, GR], BF16, kind="ExternalInput")
    wbig_d = nc.dram_tensor("wbigT", [128, 8, GR + NT], BF16, kind="ExternalInput")
    mtag_d = nc.dram_tensor("mtag", [NT, GR], BF16, kind="ExternalInput")
    bcomb_d = nc.dram_tensor("bcomb", [NT, 1], F32, kind="ExternalInput")
    wproj_d = nc.dram_tensor("wprojT", [128, 8, PR], BF16, kind="ExternalInput")
    bproj_d = nc.dram_tensor("bproj", [1, PR], F32, kind="ExternalInput")

    word_d = nc.dram_tensor("word_out", [NTOK, PR], F32, kind="ExternalOutput")
    tag_d = nc.dram_tensor("tag_out", [NT, NTOK], F32, kind="ExternalOutput")

    with tile.TileContext(nc) as tc:
        with (
            tc.tile_pool(name="wpool", bufs=1) as wpool,
            tc.tile_pool(name="emb", bufs=1) as embp,
            tc.tile_pool(name="state", bufs=1) as statep,
            tc.tile_pool(name="step", bufs=3) as stepp,
            tc.tile_pool(name="pg", bufs=2, space=bass.MemorySpace.PSUM) as pgp,
            tc.tile_pool(name="ptag", bufs=2, space=bass.MemorySpace.PSUM) as ptagp,
            tc.tile_pool(name="pw", bufs=2, space=bass.MemorySpace.PSUM) as pwp,
            tc.tile_pool(name="px", bufs=2, space=bass.MemorySpace.PSUM) as pxp,
            tc.tile_pool(name="wout", bufs=2) as woutp,
            tc.tile_pool(name="dram", bufs=4, space="DRAM") as dramp,
            tc.tile_pool(name="tago", bufs=2) as tagop,
        ):
            # ---------- load weights ----------
            wx = embp.tile([128, 4, GR], BF16, tag="wx")
            nc.sync.dma_start(wx[:], wx_d[:])
            wbig = wpool.tile([128, 8, GR + NT], BF16, tag="wbig")
            nc.sync.dma_start(wbig[:], wbig_d[:])
            mtag = wpool.tile([NT, GR], BF16, tag="mtag")
            nc.sync.dma_start(mtag[:], mtag_d[:])
            bcomb = wpool.tile([NT, 1], F32, tag="bcomb")
            nc.sync.dma_start(bcomb[:], bcomb_d[:])
            wproj = wpool.tile([128, 8, PR], BF16, tag="wproj")
            nc.sync.dma_start(wproj[:], wproj_d[:])
            bprow = wpool.tile([1, PR], F32, tag="bprow")
            nc.sync.dma_start(bprow[:], bproj_d[:])
            bprow16 = wpool.tile([1, PR], BF16, tag="bprow16")
            nc.vector.tensor_copy(bprow16[:], bprow[:])
            bproj = wpool.tile([128, PR], BF16, tag="bproj")
            nc.gpsimd.partition_broadcast(bproj[:], bprow16[:])

            ones4 = wpool.tile([NT, 1], F32, tag="ones4")
            nc.vector.memset(ones4[:], 1.0)
            ones1 = wpool.tile([1, NT], F32, tag="ones1")
            nc.vector.memset(ones1[:], 1.0)
            tag0 = wpool.tile([NT, B], BF16, tag="tag0")
            nc.vector.memset(tag0[:], 1.0 / NT)

            # ---------- X = W_x_shard @ embT  (X.T layout [GR, NTOK]) ----------
            xT = statep.tile([128, 4, NTOK], BF16, tag="xT")  # m-tile, row, token
            XN = min(512, NTOK)
            for nch in range(NTOK // XN):
                embc = embp.tile([128, 4, XN], BF16, tag="embc", name="embc",
                                 bufs=2)
                nc.sync.dma_start(embc[:], embT_d[:, :, nch * XN:(nch + 1) * XN])
                for m in range(4):
                    px = pxp.tile([128, XN], F32, tag="px", name="px")
                    for ek in range(4):
                        nc.tensor.matmul(
                            px[:],
                            wx[:, ek, m * 128:(m + 1) * 128],
                            embc[:, ek, :],
                            start=(ek == 0),
                            stop=(ek == 3),
                        )
                    nc.vector.tensor_copy(xT[:, m, nch * XN:(nch + 1) * XN], px[:])

            # ---------- recurrence state ----------
            c_sb = [statep.tile([128, B], F32, tag=f"c{i}", name=f"c{i}") for i in range(2)]
            nc.vector.memset(c_sb[0][:], 0.0)
            # gathered h chunks: [slot(8), 128, chunk*B] as [128, 8, chunk, B]
            hT = [statep.tile([128, 8, chunk, B], BF16, tag=f"hT{i}", name=f"hT{i}") for i in range(2)]
            tagn = [statep.tile([NT, B], BF16, tag=f"tagn{i}", name=f"tagn{i}") for i in range(2)]
            sig = statep.tile([128, 3 * B], F32, tag="sig")  # i,f,o post-sigmoid
            tg_t = statep.tile([128, B], F32, tag="tg")
            thc = statep.tile([128, B], F32, tag="thc")
            tmp1 = statep.tile([128, B], F32, tag="tmp1")
            tmp2 = statep.tile([128, B], F32, tag="tmp2")
            etag = statep.tile([NT, B], F32, tag="etag")
            rsum = statep.tile([1, B], F32, tag="rsum")

            def gate_psum():
                return pgp.tile([128, 4 * B], F32, tag="pg", name="pg")

            def tag_psum():
                # [NT,B] tag logits | [1,B] sum | [NT,B] bcast -- one bank
                return ptagp.tile([NT, 3 * B], F32, tag="pt", name="pt")

            def proj_chunk(k, hTk):
                """word projection for timesteps [k*chunk, (k+1)*chunk)."""
                for tsub in range(chunk * B // 128):
                    lhs = hTk.rearrange("p e t b -> p e (t b)")
                    wout = woutp.tile([128, PR], F32, tag="wout", name="wout")
                    for nch in range(PR // NB):
                        pw = pwp.tile([128, NB], F32, tag="pw", name="pw")
                        for d in range(8):
                            nc.tensor.matmul(
                                pw[:],
                                lhs[:, d, tsub * 128:(tsub + 1) * 128],
                                wproj[:, d, nch * NB:(nch + 1) * NB],
                                start=(d == 0),
                                stop=(d == 7),
                            )
                        nc.vector.tensor_add(
                            wout[:, nch * NB:(nch + 1) * NB], pw[:],
                            bproj[:, nch * NB:(nch + 1) * NB])
                    nc.sync.dma_start(
                        word_d[k * CH16 + tsub * 128:k * CH16 + (tsub + 1) * 128, :],
                        wout[:])

            tago_tiles = {}

            for t in range(SN + 1):
                last = t == SN
                pg = gate_psum() if not last else None
                pt = tag_psum() if t > 0 else None

                if t > 0:
                    # gathered h_{t-1} slots: chunk buffer and column
                    ck = ((t - 1) // chunk) % 2
                    j = (t - 1) % chunk
                    hs = hT[ck]

                    # tag logits for step t-1 (5th M tile), over gathered h
                    for k in range(8):
                        nc.tensor.matmul(
                            pt[:, 0:B],
                            wbig[:, k, GR:GR + NT],
                            hs[:, k, j, :],
                            start=(k == 0),
                            stop=(k == 7),
                        )
                    # store tag logits (+bias) to the per-chunk staging tile
                    tk = (t - 1) // chunk
                    if (t - 1) % chunk == 0:
                        tago_tiles[tk] = tagop.tile([NT, CH16], F32, tag="tago", name=f"tago{tk}")
                    nc.scalar.activation(
                        tago_tiles[tk][:, j * B:(j + 1) * B], pt[:, 0:B],
                        AF.Identity, bias=bcomb[:])
                    if (t - 1) % chunk == chunk - 1:
                        nc.sync.dma_start(
                            tag_d[:, tk * CH16:(tk + 1) * CH16], tago_tiles[tk][:])

                    if not last:
                        # softmax -> tag_norm (feeds this step's gates)
                        nc.scalar.activation(etag[:], pt[:, 0:B], AF.Exp,
                                             bias=bcomb[:])
                        nc.tensor.matmul(pt[:1, B:2 * B], ones4[:], etag[:],
                                         start=True, stop=True)
                        nc.vector.reciprocal(rsum[:], pt[:1, B:2 * B])
                        nc.tensor.matmul(pt[:, 2 * B:3 * B], ones1[:], rsum[:],
                                         start=True, stop=True)
                        nc.vector.tensor_mul(tagn[t % 2][:], etag[:],
                                             pt[:, 2 * B:3 * B])

                    if not last:
                        # gates = W_big.T tiles @ gathered h
                        for m in range(4):
                            for k in range(8):
                                nc.tensor.matmul(
                                    pg[:, m * B:(m + 1) * B],
                                    wbig[:, k, m * 128:(m + 1) * 128],
                                    hs[:, k, j, :],
                                    start=(k == 0),
                                    stop=False,
                                )
                            nc.tensor.matmul(
                                pg[:, m * B:(m + 1) * B],
                                mtag[:, m * 128:(m + 1) * 128],
                                tagn[t % 2][:],
                                start=False,
                                stop=True,
                            )
                else:
                    # t == 0: h_{-1} = 0, tag = uniform
                    for m in range(4):
                        nc.tensor.matmul(
                            pg[:, m * B:(m + 1) * B],
                            mtag[:, m * 128:(m + 1) * 128],
                            tag0[:],
                            start=True,
                            stop=True,
                        )

                if last:
                    break

                # += X_t ; activations ; state update
                nc.vector.tensor_add(pg[:, 0:B], pg[:, 0:B],
                                     xT[:, 0, t * B:(t + 1) * B])
                nc.vector.tensor_add(pg[:, B:2 * B], pg[:, B:2 * B],
                                     xT[:, 1, t * B:(t + 1) * B])
                nc.vector.tensor_add(pg[:, 2 * B:3 * B], pg[:, 2 * B:3 * B],
                                     xT[:, 2, t * B:(t + 1) * B])
                nc.vector.tensor_add(pg[:, 3 * B:4 * B], pg[:, 3 * B:4 * B],
                                     xT[:, 3, t * B:(t + 1) * B])

                nc.scalar.activation(sig[:, 0:B], pg[:, 0:B], AF.Sigmoid)      # i
                nc.scalar.activation(sig[:, B:2 * B], pg[:, B:2 * B], AF.Sigmoid)  # f
                nc.scalar.activation(tg_t[:], pg[:, 2 * B:3 * B], AF.Tanh)     # g
                nc.scalar.activation(sig[:, 2 * B:3 * B], pg[:, 3 * B:4 * B],
                                     AF.Sigmoid)                               # o

                c_prev, c_next = c_sb[t % 2], c_sb[(t + 1) % 2]
                nc.vector.tensor_mul(tmp1[:], sig[:, 0:B], tg_t[:])
                nc.vector.tensor_mul(tmp2[:], sig[:, B:2 * B], c_prev[:])
                nc.vector.tensor_add(c_next[:], tmp1[:], tmp2[:])
                nc.scalar.activation(thc[:], c_next[:], AF.Tanh)

                hpay = stepp.tile([128, B], BF16, tag="hpay", name="hpay")
                nc.vector.tensor_mul(hpay[:], sig[:, 2 * B:3 * B], thc[:])

                # ---- AllGather h_t ----
                agin = dramp.tile([128, B], BF16, tag="agin", name="agin")
                agout = dramp.tile([8, 128, B], BF16, tag="agout", name="agout")
                nc.sync.dma_start(agin[:], hpay[:])
                nc.gpsimd.collective_compute(
                    "AllGather",
                    ALU.bypass,
                    replica_groups=[list(range(NC))],
                    ins=[agin.opt()],
                    outs=[agout.rearrange("e p b -> (e p) b").opt()],
                )
                ck_w = (t // chunk) % 2
                jw = t % chunk
                nc.sync.dma_start(hT[ck_w][:, :, jw, :],
                                  agout.rearrange("e p b -> p e b"))

                # chunk complete -> projection
                if t % chunk == chunk - 1:
                    proj_chunk(t // chunk, hT[ck_w])

    nc.compile()
    return nc


def _prep_inputs(input_seq, token_embedding, W_ih, W_hh, W_tag, b_tag,
                 W_lh, b_lh, W_proj, b_proj, s_steps=S):
    """Host-side sharding/layout. Returns in_maps (list of 8 dicts)."""
    SN = s_steps
    NTOK = SN * B
    f32 = np.float32

    input_seq = np.asarray(input_seq)
    token_embedding = np.asarray(token_embedding, f32)
    W_ih = np.asarray(W_ih, f32)
    W_hh = np.asarray(W_hh, f32)
    W_tag = np.asarray(W_tag, f32)
    b_tag = np.asarray(b_tag, f32)
    W_lh = np.asarray(W_lh, f32)
    b_lh = np.asarray(b_lh, f32)
    W_proj = np.asarray(W_proj, f32)
    b_proj = np.asarray(b_proj, f32)

    # emb.T in (t, b) token order: [E_TOK, NTOK]
    emb = token_embedding[input_seq[:, :SN]]            # [B, SN, E]
    embT = np.ascontiguousarray(emb.transpose(2, 1, 0).reshape(E_TOK, NTOK))
    embT_t = np.ascontiguousarray(embT.reshape(4, 128, NTOK).transpose(1, 0, 2)).astype(bf16)

    # weight folds (weight-only transforms)
    W_comb = W_tag @ W_lh                               # [NT, H]
    b_comb = (b_lh @ W_tag.T + b_tag).reshape(NT, 1)    # [NT, 1]
    M_tag_full = W_tag @ W_ih[:, E_TOK:].T              # [NT, 4H]
    W_projf = W_proj.reshape(NT * TV, H)
    b_projf = b_proj.reshape(NT * TV)

    in_maps = []
    for c in range(NC):
        rows = np.concatenate([np.arange(g * H + c * 128, g * H + (c + 1) * 128)
                               for g in range(4)])
        wx = W_ih[rows, :E_TOK]                         # [GR, E]
        wxT = np.ascontiguousarray(wx.T.reshape(4, 128, GR).transpose(1, 0, 2)).astype(bf16)
        wbig = np.concatenate([W_hh[rows, :], W_comb], axis=0)  # [GR+NT, H]
        wbigT = np.ascontiguousarray(wbig.T.reshape(8, 128, GR + NT).transpose(1, 0, 2)).astype(bf16)
        mtag = M_tag_full[:, rows].astype(bf16)         # [NT, GR]
        wp = W_projf[c * PR:(c + 1) * PR, :]            # [PR, H]
        wpT = np.ascontiguousarray(wp.T.reshape(8, 128, PR).transpose(1, 0, 2)).astype(bf16)
        bp = b_projf[c * PR:(c + 1) * PR].reshape(1, PR).astype(f32)
        in_maps.append({
            "embT": embT_t,
            "wxT": wxT,
            "wbigT": wbigT,
            "mtag": np.ascontiguousarray(mtag),
            "bcomb": np.ascontiguousarray(b_comb),
            "wprojT": wpT,
            "bproj": bp,
        })
    return in_maps


def _assemble(results, s_steps=S):
    SN = s_steps
    f32 = np.float32
    tag = results[0]["tag_out"]                         # [NT, SN*B]
    tag_logits = np.ascontiguousarray(
        tag.reshape(NT, SN, B).transpose(2, 1, 0)).astype(f32)   # [B, SN, NT]
    word_logits = np.empty((NT, B, SN, TV), f32)
    for c in range(NC):
        w = results[c]["word_out"]                      # [SN*B, PR]
        part = w.reshape(SN, B, PR).transpose(1, 0, 2)  # [B, SN, PR]
        t_i, v_i = divmod(c * PR, TV)
        word_logits[t_i, :, :, v_i:v_i + PR] = part
    return tag_logits, word_logits


_CACHED = {}


def kernel(input_seq, token_embedding, W_ih, W_hh, W_tag, b_tag, W_lh, b_lh,
           W_proj, b_proj, s_steps=S, trace=False):
    key = (s_steps,)
    if key not in _CACHED:
        _CACHED[key] = build_kernel(s_steps)
    nc = _CACHED[key]
    in_maps = _prep_inputs(input_seq, token_embedding, W_ih, W_hh, W_tag,
                           b_tag, W_lh, b_lh, W_proj, b_proj, s_steps)
    res = run_bass_kernel_spmd(nc, in_maps, core_ids=list(range(NC)),
                               trace=trace)
    out = _assemble(res.results, s_steps)
    if trace:
        kernel.last_exec_time_ns = res.exec_time_ns
        kernel.last_trace = res.instructions_and_trace
    return out
